# revision 16
# baseline (speedup 1.0000x reference)
"""KDE on a 20^3 grid, distributed across 8 TRN2 NeuronCores.

Separable-factorization algorithm (replaces the dense 8000x8000 kernel
matrix): with A = cov_inv, q_v = v^T A v, and centered grid coords,

  kde[i] = sum_b p_b exp(-0.5(q_i + q_b - 2 GA_i . x_b))
         = e^{F_i} * sum_{x1,x2,x3} w'[x1,x2,x3] E1[x1,i] E2[x2,i] E3[x3,i]

since GA_i . x_b = sum_k g_k,i x_k,b factorizes over the tensor-product
grid.  Per-axis tables absorb t_k(x) = 0.5 A_kk x^2 (so the b-side
factor w' = p * exp(-(cross terms)) stays inside fp32 range) and per-i
shifts s_k,i = max(0, 9.5|g_k,i| - 30) (so E-table entries and partial
sums stay inside fp32 range); F_i = -0.5 q_i + sum_k s_k,i compensates.

Device pipeline per core (1000 query rows i, full b-grid, 8 i-chunks):
  - 3 packed input DMAs; exponent tiles built by tiny fp16 hi/lo
    matmuls (1-pass PE), ScalarE exponentiates to bf16/f32.
  - stage 1 is ONE bf16 matmul per chunk (k=20):
    out1[i, x12] = sum_x3 E3[x3,i] w'[x3,x12] into PSUM.
  - E12 = E1 (x) E2 prebuilt per chunk on GpSimd (SBUF-only engine).
  - main loop: PE matmul -> one DVE scalar_tensor_tensor that reads
    PSUM, multiplies by E12, and free-dim-accumulates into the kde
    column (accum_out).
  - final: multiply by e^{F}, 32x32 block-transpose, one DMA out.
Normalization (a global scalar) happens on the host after gathering.
"""

import numpy as np

GRID = (20, 20, 20)
N = 8000
NCORES = 8
ROWS = N // NCORES          # 1000 query rows per core
NCH = 8                     # i-chunks of 128 (last 24 cols are padding)
NI = NCH * 128              # 1024 padded rows per core
SHIFT_B = 30.0              # per-axis shift budget

# pats (fp16) column layout
_GS0 = 0                    # gscol10 [10, 1024]
_G3 = 1024                  # g3row6  [6, 1024]
_ETP = 2048                 # etpat10 [10, 40]
_E3P = 2088                 # e3pat6  [6, 20]
_PATW = 2108

_PROGRAM = None


def _build_program(num_devices=NCORES):
    from contextlib import ExitStack

    import concourse.bacc as bacc
    import concourse.mybir as mybir
    import concourse.tile as tile

    f32 = mybir.dt.float32
    f16 = mybir.dt.float16
    bf16 = mybir.dt.bfloat16
    OP = mybir.AluOpType
    EXP = mybir.ActivationFunctionType.Exp

    nc = bacc.Bacc(
        "TRN2",
        target_bir_lowering=False,
        debug=False,
        num_devices=num_devices,
    )

    pw_d = nc.dram_tensor("pw", [20, 800], f32, kind="ExternalInput").ap()
    pats_d = nc.dram_tensor("pats", [10, _PATW], f16, kind="ExternalInput").ap()
    fmat_d = nc.dram_tensor("fmat", [128, NCH], f32, kind="ExternalInput").ap()
    out_d = nc.dram_tensor("out", [8, 128], f32, kind="ExternalOutput").ap()

    with tile.TileContext(nc) as tc, ExitStack() as ctx:
        const = ctx.enter_context(tc.tile_pool(name="const", bufs=1))
        work = ctx.enter_context(tc.tile_pool(name="work", bufs=3))
        psum_pre = ctx.enter_context(
            tc.tile_pool(name="psum_pre", bufs=1, space="PSUM")
        )

        # ---- input loads (3 packed DMAs, dispatched from idle engines) ----
        pats_sb = const.tile([10, _PATW], f16)
        nc.sync.dma_start(out=pats_sb[:], in_=pats_d[:])
        pw_sb = const.tile([20, 800], f32)
        nc.gpsimd.dma_start(out=pw_sb[:], in_=pw_d[:])
        fmat_sb = const.tile([128, NCH], f32)
        nc.scalar.dma_start(out=fmat_sb[:], in_=fmat_d[:])

        # ---- w' = p * Wfac -> bf16 (one fused vector op) ----
        whi = const.tile([20, 400], bf16)
        nc.vector.scalar_tensor_tensor(
            whi[:], pw_sb[:, 0:400], 1.0, pw_sb[:, 400:800],
            op0=OP.mult, op1=OP.mult,
        )

        # ---- E3 [x3, i]: fp16 k=6 exponent matmuls -> Exp -> bf16 ----
        e3s = const.tile([20, NI], bf16)
        for h in range(2):
            sl = slice(h * 512, (h + 1) * 512)
            xp3 = psum_pre.tile([20, 512], f32, tag="xp3", bufs=2)
            nc.tensor.matmul(
                xp3[:],
                lhsT=pats_sb[0:6, _E3P : _E3P + 20],
                rhs=pats_sb[0:6, _G3 + h * 512 : _G3 + h * 512 + 512],
                start=True,
                stop=True,
            )
            nc.scalar.activation(e3s[:, sl], xp3[:], EXP)

        # ---- E1/E2 exponents: fp16 k=10 matmuls, packed [128, 8*40] ----
        xpe = psum_pre.tile([128, NCH * 40], f32)
        for ci in range(NCH):
            nc.tensor.matmul(
                xpe[:, ci * 40 : ci * 40 + 40],
                lhsT=pats_sb[:, ci * 128 : (ci + 1) * 128],
                rhs=pats_sb[:, _ETP : _ETP + 40],
                start=True,
                stop=True,
            )
        et = const.tile([128, NCH * 40], f32)
        for qq in range(4):
            nc.scalar.activation(
                et[:, qq * 80 : (qq + 1) * 80], xpe[:, qq * 80 : (qq + 1) * 80], EXP
            )

        # ---- e^{F} ----
        ef = const.tile([128, NCH], f32)
        nc.scalar.activation(ef[:], fmat_sb[:], EXP)

        # ---- E12 = E1 (x) E2, bf16, built on GpSimd (keeps DVE free) ----
        e12 = const.tile([128, NCH * 400], bf16)
        for ci in range(NCH):
            e1b = (
                et[:, ci * 40 : ci * 40 + 20]
                .unsqueeze(2)
                .broadcast_to((128, 20, 20))
            )
            e2b = (
                et[:, ci * 40 + 20 : ci * 40 + 40]
                .unsqueeze(1)
                .broadcast_to((128, 20, 20))
            )
            e12v = e12[:, ci * 400 : ci * 400 + 400].rearrange(
                "p (a b) -> p a b", a=20, b=20
            )
            eng = nc.vector if ci < 4 else nc.gpsimd
            eng.tensor_mul(e12v, e1b, e2b)


        # ---- main loop over 8 i-chunks ----
        kdeT = const.tile([128, NCH], f32)
        for ci in range(NCH):
            isl = slice(ci * 128, (ci + 1) * 128)
            o1p = psum_pre.tile([128, 400], f32, tag="o1p", bufs=4)
            nc.tensor.matmul(
                o1p[:], lhsT=e3s[:, isl], rhs=whi[:], start=True, stop=True
            )
            prod = work.tile([128, 400], bf16, tag="prod")
            nc.vector.scalar_tensor_tensor(
                prod[:],
                o1p[:],
                1.0,
                e12[:, ci * 400 : ci * 400 + 400],
                op0=OP.mult,
                op1=OP.mult,
                accum_out=kdeT[:, ci : ci + 1],
            )

        # ---- scale by e^{F}, transpose to row-major i order, DMA out ----
        ksc = const.tile([128, 32], f32)
        nc.vector.memset(ksc[:], 0.0)
        nc.vector.tensor_mul(ksc[:, 0:NCH], kdeT[:], ef[:])
        t32 = const.tile([128, 32], f32)
        nc.vector.transpose(t32[:], ksc[:])
        kout = const.tile([32, 128], f32)
        for r in range(4):
            nc.vector.tensor_copy(
                kout[0:8, r * 32 : (r + 1) * 32], t32[r * 32 : r * 32 + 8, 0:32]
            )
        nc.sync.dma_start(out=out_d[:], in_=kout[0:8, :])

    nc.compile()
    return nc


def _get_program():
    global _PROGRAM
    if _PROGRAM is None:
        _PROGRAM = _build_program()
    return _PROGRAM


def _split16(v):
    hi = v.astype(np.float16).astype(np.float64)
    return hi, v - hi


def _host_inputs(space_probs, cov_inv):
    """Per-core input maps: host-side layout + coordinate-table prep."""
    p = np.asarray(space_probs, dtype=np.float64).reshape(-1)
    A = np.asarray(cov_inv, dtype=np.float64)

    idx = np.indices(GRID, dtype=np.float64).reshape(3, N)
    cc = idx - 9.5                        # centered coords, [3, N]
    c20 = np.arange(20, dtype=np.float64) - 9.5

    G = (cc.T @ A).T                      # [3, N] g_k,i
    q = np.sum(cc * G, axis=0)            # [N]
    s = np.maximum(0.0, 9.5 * np.abs(G) - SHIFT_B)   # [3, N]
    F = -0.5 * q + s.sum(axis=0)          # [N]
    t = 0.5 * np.diag(A)[:, None] * (c20**2)[None, :]  # [3, 20]

    crossexp = -(
        A[0, 1] * cc[0] * cc[1] + A[0, 2] * cc[0] * cc[2] + A[1, 2] * cc[1] * cc[2]
    )
    wfac = np.exp(crossexp).reshape(400, 20).T
    pt = p.reshape(400, 20).T
    pw = np.zeros((20, 800), dtype=np.float32)
    pw[:, 0:400] = pt
    pw[:, 400:800] = wfac

    th = [_split16(t[k]) for k in range(3)]

    # etpat10 rows: [g1h,g1l,g2h,g2l,s1h,s1l,s2h,s2l,1(t hi),1(t lo)]
    etpat = np.zeros((10, 40), dtype=np.float16)
    etpat[0, 0:20] = c20
    etpat[1, 0:20] = c20
    etpat[4, 0:20] = -1.0
    etpat[5, 0:20] = -1.0
    etpat[8, 0:20] = -th[0][0]
    etpat[9, 0:20] = -th[0][1]
    etpat[2, 20:40] = c20
    etpat[3, 20:40] = c20
    etpat[6, 20:40] = -1.0
    etpat[7, 20:40] = -1.0
    etpat[8, 20:40] = -th[1][0]
    etpat[9, 20:40] = -th[1][1]

    # e3pat6 rows: [g3h, g3l, s3h, s3l, 1(t hi), 1(t lo)]
    e3pat = np.zeros((6, 20), dtype=np.float16)
    e3pat[0] = c20
    e3pat[1] = c20
    e3pat[2] = -1.0
    e3pat[3] = -1.0
    e3pat[4] = -th[2][0]
    e3pat[5] = -th[2][1]

    in_maps = []
    for r in range(NCORES):
        i0 = r * ROWS
        sl = slice(i0, i0 + ROWS)

        pats = np.zeros((10, _PATW), dtype=np.float16)
        hi_rows = (0, 2, 4, 6)
        lo_rows = (1, 3, 5, 7)
        for k, src in enumerate((G[0], G[1], s[0], s[1])):
            hi, lo = _split16(src[sl])
            pats[hi_rows[k], _GS0 : _GS0 + ROWS] = hi
            pats[lo_rows[k], _GS0 : _GS0 + ROWS] = lo
        pats[8, _GS0 : _GS0 + ROWS] = 1.0
        pats[9, _GS0 : _GS0 + ROWS] = 1.0

        g3h, g3l = _split16(G[2][sl])
        s3h, s3l = _split16(s[2][sl])
        pats[0, _G3 : _G3 + ROWS] = g3h
        pats[1, _G3 : _G3 + ROWS] = g3l
        pats[2, _G3 : _G3 + ROWS] = s3h
        pats[3, _G3 : _G3 + ROWS] = s3l
        pats[4, _G3 : _G3 + ROWS] = 1.0
        pats[5, _G3 : _G3 + ROWS] = 1.0

        pats[:, _ETP : _ETP + 40] = etpat
        pats[0:6, _E3P : _E3P + 20] = e3pat

        fm = np.zeros((NCH, 128), dtype=np.float32)
        fm.reshape(-1)[:ROWS] = F[sl]
        fmat = np.ascontiguousarray(fm.T)

        in_maps.append({"pw": pw, "pats": pats, "fmat": fmat})
    return in_maps


def kernel(space_probs, cov_inv):
    from concourse.bass_utils import run_bass_kernel_spmd

    nc = _get_program()
    in_maps = _host_inputs(space_probs, cov_inv)
    res = run_bass_kernel_spmd(nc, in_maps, list(range(NCORES)))
    out = np.concatenate(
        [res.results[r]["out"].reshape(-1)[:ROWS] for r in range(NCORES)]
    )
    out = out / out.sum(dtype=np.float64)
    return out.reshape(GRID).astype(np.float32)


# revision 17
# speedup vs baseline: 1.0224x; 1.0224x over previous
"""KDE on a 20^3 grid, distributed across 8 TRN2 NeuronCores.

Separable-factorization algorithm (replaces the dense 8000x8000 kernel
matrix): with A = cov_inv, q_v = v^T A v, and centered grid coords,

  kde[i] = sum_b p_b exp(-0.5(q_i + q_b - 2 GA_i . x_b))
         = e^{F_i} * sum_{x1,x2,x3} w'[x1,x2,x3] E1[x1,i] E2[x2,i] E3[x3,i]

since GA_i . x_b = sum_k g_k,i x_k,b factorizes over the tensor-product
grid.  Per-axis tables absorb t_k(x) = 0.5 A_kk x^2 (so the b-side
factor w' = p * exp(-(cross terms)) stays inside fp32 range) and per-i
shifts s_k,i = max(0, 9.5|g_k,i| - 30) (so E-table entries and partial
sums stay inside fp32 range); F_i = -0.5 q_i + sum_k s_k,i compensates.

Device pipeline per core (1000 query rows i, full b-grid, 8 i-chunks):
  - 3 packed input DMAs; exponent tiles built by tiny fp16 hi/lo
    matmuls (1-pass PE), ScalarE exponentiates to bf16/f32.
  - stage 1 is ONE bf16 matmul per chunk (k=20):
    out1[i, x12] = sum_x3 E3[x3,i] w'[x3,x12] into PSUM.
  - E12 = E1 (x) E2 prebuilt per chunk on GpSimd (SBUF-only engine).
  - main loop: PE matmul -> one DVE scalar_tensor_tensor that reads
    PSUM, multiplies by E12, and free-dim-accumulates into the kde
    column (accum_out).
  - final: multiply by e^{F}, 32x32 block-transpose, one DMA out.
Normalization (a global scalar) happens on the host after gathering.
"""

import numpy as np

GRID = (20, 20, 20)
N = 8000
NCORES = 8
ROWS = N // NCORES          # 1000 query rows per core
NCH = 8                     # i-chunks of 128 (last 24 cols are padding)
NI = NCH * 128              # 1024 padded rows per core
SHIFT_B = 30.0              # per-axis shift budget

# pats (fp16) column layout
_GS0 = 0                    # gscol10 [10, 1024]
_G3 = 1024                  # g3row6  [6, 1024]
_ETP = 2048                 # etpat10 [10, 40]
_E3P = 2088                 # e3pat6  [6, 20]
_PATW = 2108

_PROGRAM = None


def _build_program(num_devices=NCORES):
    from contextlib import ExitStack

    import concourse.bacc as bacc
    import concourse.mybir as mybir
    import concourse.tile as tile

    f32 = mybir.dt.float32
    f16 = mybir.dt.float16
    bf16 = mybir.dt.bfloat16
    OP = mybir.AluOpType
    EXP = mybir.ActivationFunctionType.Exp

    nc = bacc.Bacc(
        "TRN2",
        target_bir_lowering=False,
        debug=False,
        num_devices=num_devices,
    )

    pw_d = nc.dram_tensor("pw", [20, 800], f32, kind="ExternalInput").ap()
    pats_d = nc.dram_tensor("pats", [10, _PATW], f16, kind="ExternalInput").ap()
    fmat_d = nc.dram_tensor("fmat", [128, NCH], f32, kind="ExternalInput").ap()
    out_d = nc.dram_tensor("out", [8, 128], f32, kind="ExternalOutput").ap()

    with tile.TileContext(nc) as tc, ExitStack() as ctx:
        const = ctx.enter_context(tc.tile_pool(name="const", bufs=1))
        work = ctx.enter_context(tc.tile_pool(name="work", bufs=3))
        psum_pre = ctx.enter_context(
            tc.tile_pool(name="psum_pre", bufs=1, space="PSUM")
        )

        # ---- input loads (3 packed DMAs, dispatched from idle engines) ----
        pats_sb = const.tile([10, _PATW], f16)
        nc.sync.dma_start(out=pats_sb[:], in_=pats_d[:])
        pw_sb = const.tile([20, 800], f32)
        nc.gpsimd.dma_start(out=pw_sb[:], in_=pw_d[:])
        fmat_sb = const.tile([128, NCH], f32)
        nc.scalar.dma_start(out=fmat_sb[:], in_=fmat_d[:])

        # ---- w' = p * Wfac -> bf16 (one fused vector op) ----
        whi = const.tile([20, 400], bf16)
        nc.vector.scalar_tensor_tensor(
            whi[:], pw_sb[:, 0:400], 1.0, pw_sb[:, 400:800],
            op0=OP.mult, op1=OP.mult,
        )

        # ---- E3 [x3, i]: fp16 k=6 exponent matmuls -> Exp -> bf16 ----
        e3s = const.tile([20, NI], bf16)
        for h in range(2):
            sl = slice(h * 512, (h + 1) * 512)
            xp3 = psum_pre.tile([20, 512], f32, tag="xp3", bufs=2)
            nc.tensor.matmul(
                xp3[:],
                lhsT=pats_sb[0:6, _E3P : _E3P + 20],
                rhs=pats_sb[0:6, _G3 + h * 512 : _G3 + h * 512 + 512],
                start=True,
                stop=True,
            )
            nc.scalar.activation(e3s[:, sl], xp3[:], EXP)

        # ---- E1/E2 exponents: fp16 k=10 matmuls, packed [128, 8*40] ----
        xpe = psum_pre.tile([128, NCH * 40], f32)
        for ci in range(NCH):
            nc.tensor.matmul(
                xpe[:, ci * 40 : ci * 40 + 40],
                lhsT=pats_sb[:, ci * 128 : (ci + 1) * 128],
                rhs=pats_sb[:, _ETP : _ETP + 40],
                start=True,
                stop=True,
            )
        et = const.tile([128, NCH * 40], f32)
        nc.scalar.activation(et[:, 0 : 4 * 40], xpe[:, 0 : 4 * 40], EXP)
        nc.scalar.activation(et[:, 4 * 40 : 8 * 40], xpe[:, 4 * 40 : 8 * 40], EXP)

        # ---- e^{F} ----
        ef = const.tile([128, NCH], f32)
        nc.scalar.activation(ef[:], fmat_sb[:], EXP)

        # ---- E12 = E1 (x) E2, bf16, built on GpSimd (keeps DVE free) ----
        e12 = const.tile([128, NCH * 400], bf16)
        for ci in range(NCH):
            e1b = (
                et[:, ci * 40 : ci * 40 + 20]
                .unsqueeze(2)
                .broadcast_to((128, 20, 20))
            )
            e2b = (
                et[:, ci * 40 + 20 : ci * 40 + 40]
                .unsqueeze(1)
                .broadcast_to((128, 20, 20))
            )
            e12v = e12[:, ci * 400 : ci * 400 + 400].rearrange(
                "p (a b) -> p a b", a=20, b=20
            )
            eng = nc.vector if ci < 3 else nc.gpsimd
            eng.tensor_mul(e12v, e1b, e2b)


        # ---- main loop over 8 i-chunks ----
        kdeT = const.tile([128, NCH], f32)
        for ci in range(NCH):
            isl = slice(ci * 128, (ci + 1) * 128)
            o1p = psum_pre.tile([128, 400], f32, tag="o1p", bufs=4)
            nc.tensor.matmul(
                o1p[:], lhsT=e3s[:, isl], rhs=whi[:], start=True, stop=True
            )
            prod = work.tile([128, 400], bf16, tag="prod")
            nc.vector.scalar_tensor_tensor(
                prod[:],
                o1p[:],
                1.0,
                e12[:, ci * 400 : ci * 400 + 400],
                op0=OP.mult,
                op1=OP.mult,
                accum_out=kdeT[:, ci : ci + 1],
            )

        # ---- scale by e^{F}, transpose to row-major i order, DMA out ----
        ksc = const.tile([128, 32], f32)
        nc.vector.memset(ksc[:], 0.0)
        nc.vector.tensor_mul(ksc[:, 0:NCH], kdeT[:], ef[:])
        t32 = const.tile([128, 32], f32)
        nc.vector.transpose(t32[:], ksc[:])
        kout = const.tile([32, 128], f32)
        for r in range(4):
            nc.vector.tensor_copy(
                kout[0:8, r * 32 : (r + 1) * 32], t32[r * 32 : r * 32 + 8, 0:32]
            )
        nc.sync.dma_start(out=out_d[:], in_=kout[0:8, :])

    nc.compile()
    return nc


def _get_program():
    global _PROGRAM
    if _PROGRAM is None:
        _PROGRAM = _build_program()
    return _PROGRAM


def _split16(v):
    hi = v.astype(np.float16).astype(np.float64)
    return hi, v - hi


def _host_inputs(space_probs, cov_inv):
    """Per-core input maps: host-side layout + coordinate-table prep."""
    p = np.asarray(space_probs, dtype=np.float64).reshape(-1)
    A = np.asarray(cov_inv, dtype=np.float64)

    idx = np.indices(GRID, dtype=np.float64).reshape(3, N)
    cc = idx - 9.5                        # centered coords, [3, N]
    c20 = np.arange(20, dtype=np.float64) - 9.5

    G = (cc.T @ A).T                      # [3, N] g_k,i
    q = np.sum(cc * G, axis=0)            # [N]
    s = np.maximum(0.0, 9.5 * np.abs(G) - SHIFT_B)   # [3, N]
    F = -0.5 * q + s.sum(axis=0)          # [N]
    t = 0.5 * np.diag(A)[:, None] * (c20**2)[None, :]  # [3, 20]

    crossexp = -(
        A[0, 1] * cc[0] * cc[1] + A[0, 2] * cc[0] * cc[2] + A[1, 2] * cc[1] * cc[2]
    )
    wfac = np.exp(crossexp).reshape(400, 20).T
    pt = p.reshape(400, 20).T
    pw = np.zeros((20, 800), dtype=np.float32)
    pw[:, 0:400] = pt
    pw[:, 400:800] = wfac

    th = [_split16(t[k]) for k in range(3)]

    # etpat10 rows: [g1h,g1l,g2h,g2l,s1h,s1l,s2h,s2l,1(t hi),1(t lo)]
    etpat = np.zeros((10, 40), dtype=np.float16)
    etpat[0, 0:20] = c20
    etpat[1, 0:20] = c20
    etpat[4, 0:20] = -1.0
    etpat[5, 0:20] = -1.0
    etpat[8, 0:20] = -th[0][0]
    etpat[9, 0:20] = -th[0][1]
    etpat[2, 20:40] = c20
    etpat[3, 20:40] = c20
    etpat[6, 20:40] = -1.0
    etpat[7, 20:40] = -1.0
    etpat[8, 20:40] = -th[1][0]
    etpat[9, 20:40] = -th[1][1]

    # e3pat6 rows: [g3h, g3l, s3h, s3l, 1(t hi), 1(t lo)]
    e3pat = np.zeros((6, 20), dtype=np.float16)
    e3pat[0] = c20
    e3pat[1] = c20
    e3pat[2] = -1.0
    e3pat[3] = -1.0
    e3pat[4] = -th[2][0]
    e3pat[5] = -th[2][1]

    in_maps = []
    for r in range(NCORES):
        i0 = r * ROWS
        sl = slice(i0, i0 + ROWS)

        pats = np.zeros((10, _PATW), dtype=np.float16)
        hi_rows = (0, 2, 4, 6)
        lo_rows = (1, 3, 5, 7)
        for k, src in enumerate((G[0], G[1], s[0], s[1])):
            hi, lo = _split16(src[sl])
            pats[hi_rows[k], _GS0 : _GS0 + ROWS] = hi
            pats[lo_rows[k], _GS0 : _GS0 + ROWS] = lo
        pats[8, _GS0 : _GS0 + ROWS] = 1.0
        pats[9, _GS0 : _GS0 + ROWS] = 1.0

        g3h, g3l = _split16(G[2][sl])
        s3h, s3l = _split16(s[2][sl])
        pats[0, _G3 : _G3 + ROWS] = g3h
        pats[1, _G3 : _G3 + ROWS] = g3l
        pats[2, _G3 : _G3 + ROWS] = s3h
        pats[3, _G3 : _G3 + ROWS] = s3l
        pats[4, _G3 : _G3 + ROWS] = 1.0
        pats[5, _G3 : _G3 + ROWS] = 1.0

        pats[:, _ETP : _ETP + 40] = etpat
        pats[0:6, _E3P : _E3P + 20] = e3pat

        fm = np.zeros((NCH, 128), dtype=np.float32)
        fm.reshape(-1)[:ROWS] = F[sl]
        fmat = np.ascontiguousarray(fm.T)

        in_maps.append({"pw": pw, "pats": pats, "fmat": fmat})
    return in_maps


def kernel(space_probs, cov_inv):
    from concourse.bass_utils import run_bass_kernel_spmd

    nc = _get_program()
    in_maps = _host_inputs(space_probs, cov_inv)
    res = run_bass_kernel_spmd(nc, in_maps, list(range(NCORES)))
    out = np.concatenate(
        [res.results[r]["out"].reshape(-1)[:ROWS] for r in range(NCORES)]
    )
    out = out / out.sum(dtype=np.float64)
    return out.reshape(GRID).astype(np.float32)


# revision 18
# speedup vs baseline: 1.0356x; 1.0129x over previous
"""KDE on a 20^3 grid, distributed across 8 TRN2 NeuronCores.

Separable-factorization algorithm (replaces the dense 8000x8000 kernel
matrix): with A = cov_inv, q_v = v^T A v, and centered grid coords,

  kde[i] = sum_b p_b exp(-0.5(q_i + q_b - 2 GA_i . x_b))
         = e^{F_i} * sum_{x1,x2,x3} w'[x1,x2,x3] E1[x1,i] E2[x2,i] E3[x3,i]

since GA_i . x_b = sum_k g_k,i x_k,b factorizes over the tensor-product
grid.  Per-axis tables absorb t_k(x) = 0.5 A_kk x^2 (so the b-side
factor w' = p * exp(-(cross terms)) stays inside fp32 range) and per-i
shifts s_k,i = max(0, 9.5|g_k,i| - 30) (so E-table entries and partial
sums stay inside fp32 range); F_i = -0.5 q_i + sum_k s_k,i compensates.

Device pipeline per core (1000 query rows i, full b-grid, 8 i-chunks):
  - 3 packed input DMAs; exponent tiles built by tiny fp16 hi/lo
    matmuls (1-pass PE), ScalarE exponentiates to bf16/f32.
  - stage 1 is ONE bf16 matmul per chunk (k=20):
    out1[i, x12] = sum_x3 E3[x3,i] w'[x3,x12] into PSUM.
  - E12 = E1 (x) E2 prebuilt per chunk on GpSimd (SBUF-only engine).
  - main loop: PE matmul -> one DVE scalar_tensor_tensor that reads
    PSUM, multiplies by E12, and free-dim-accumulates into the kde
    column (accum_out).
  - final: multiply by e^{F}, 32x32 block-transpose, one DMA out.
Normalization (a global scalar) happens on the host after gathering.
"""

import numpy as np

GRID = (20, 20, 20)
N = 8000
NCORES = 8
ROWS = N // NCORES          # 1000 query rows per core
NCH = 8                     # i-chunks of 128 (last 24 cols are padding)
NI = NCH * 128              # 1024 padded rows per core
SHIFT_B = 30.0              # per-axis shift budget

# pats (fp16) column layout
_GS0 = 0                    # gscol10 [10, 1024]
_G3 = 1024                  # g3row6  [6, 1024]
_ETP = 2048                 # etpat10 [10, 40]
_E3P = 2088                 # e3pat6  [6, 20]
_PATW = 2108

_PROGRAM = None


def _build_program(num_devices=NCORES):
    from contextlib import ExitStack

    import concourse.bacc as bacc
    import concourse.mybir as mybir
    import concourse.tile as tile

    f32 = mybir.dt.float32
    f16 = mybir.dt.float16
    bf16 = mybir.dt.bfloat16
    OP = mybir.AluOpType
    EXP = mybir.ActivationFunctionType.Exp

    nc = bacc.Bacc(
        "TRN2",
        target_bir_lowering=False,
        debug=False,
        num_devices=num_devices,
    )

    pw_d = nc.dram_tensor("pw", [20, 800], f32, kind="ExternalInput").ap()
    pats_d = nc.dram_tensor("pats", [10, _PATW], f16, kind="ExternalInput").ap()
    fmat_d = nc.dram_tensor("fmat", [128, NCH], f32, kind="ExternalInput").ap()
    out_d = nc.dram_tensor("out", [8, 128], f32, kind="ExternalOutput").ap()

    with tile.TileContext(nc) as tc, ExitStack() as ctx:
        const = ctx.enter_context(tc.tile_pool(name="const", bufs=1))
        work = ctx.enter_context(tc.tile_pool(name="work", bufs=3))
        psum_pre = ctx.enter_context(
            tc.tile_pool(name="psum_pre", bufs=1, space="PSUM")
        )

        # ---- input loads (3 packed DMAs, dispatched from idle engines) ----
        pats_sb = const.tile([10, _PATW], f16)
        nc.sync.dma_start(out=pats_sb[:], in_=pats_d[:])
        pw_sb = const.tile([20, 800], f32)
        nc.gpsimd.dma_start(out=pw_sb[:], in_=pw_d[:])
        fmat_sb = const.tile([128, NCH], f32)
        nc.scalar.dma_start(out=fmat_sb[:], in_=fmat_d[:])

        # ---- w' = p * Wfac -> bf16 (one fused vector op) ----
        whi = const.tile([20, 400], bf16)
        nc.vector.scalar_tensor_tensor(
            whi[:], pw_sb[:, 0:400], 1.0, pw_sb[:, 400:800],
            op0=OP.mult, op1=OP.mult,
        )

        # ---- E3 [x3, i]: fp16 k=6 exponent matmuls -> Exp -> bf16 ----
        e3s = const.tile([20, NI], bf16)
        for h in range(2):
            sl = slice(h * 512, (h + 1) * 512)
            xp3 = psum_pre.tile([20, 512], f32, tag="xp3", bufs=2)
            nc.tensor.matmul(
                xp3[:],
                lhsT=pats_sb[0:6, _E3P : _E3P + 20],
                rhs=pats_sb[0:6, _G3 + h * 512 : _G3 + h * 512 + 512],
                start=True,
                stop=True,
            )
            nc.scalar.activation(e3s[:, sl], xp3[:], EXP)

        # ---- E1/E2 exponents: fp16 k=10 matmuls, packed [128, 8*40] ----
        xpe = psum_pre.tile([128, NCH * 40], f32)
        for ci in range(NCH):
            nc.tensor.matmul(
                xpe[:, ci * 40 : ci * 40 + 40],
                lhsT=pats_sb[:, ci * 128 : (ci + 1) * 128],
                rhs=pats_sb[:, _ETP : _ETP + 40],
                start=True,
                stop=True,
            )
        et = const.tile([128, NCH * 40], f32)
        nc.scalar.activation(et[:, 0 : 4 * 40], xpe[:, 0 : 4 * 40], EXP)
        nc.scalar.activation(et[:, 4 * 40 : 8 * 40], xpe[:, 4 * 40 : 8 * 40], EXP)

        # ---- e^{F} ----
        ef = const.tile([128, NCH], f32)
        nc.scalar.activation(ef[:], fmat_sb[:], EXP)

        # ---- E12 = E1 (x) E2, bf16, built on GpSimd (keeps DVE free) ----
        e12 = const.tile([128, NCH * 400], bf16)
        for ci in range(NCH):
            e1b = (
                et[:, ci * 40 : ci * 40 + 20]
                .unsqueeze(2)
                .broadcast_to((128, 20, 20))
            )
            e2b = (
                et[:, ci * 40 + 20 : ci * 40 + 40]
                .unsqueeze(1)
                .broadcast_to((128, 20, 20))
            )
            e12v = e12[:, ci * 400 : ci * 400 + 400].rearrange(
                "p (a b) -> p a b", a=20, b=20
            )
            eng = nc.vector if ci < 2 else nc.gpsimd
            eng.tensor_mul(e12v, e1b, e2b)


        # ---- main loop over 8 i-chunks ----
        kdeT = const.tile([128, NCH], f32)
        for ci in range(NCH):
            isl = slice(ci * 128, (ci + 1) * 128)
            o1p = psum_pre.tile([128, 400], f32, tag="o1p", bufs=4)
            nc.tensor.matmul(
                o1p[:], lhsT=e3s[:, isl], rhs=whi[:], start=True, stop=True
            )
            prod = work.tile([128, 400], bf16, tag="prod")
            nc.vector.scalar_tensor_tensor(
                prod[:],
                o1p[:],
                1.0,
                e12[:, ci * 400 : ci * 400 + 400],
                op0=OP.mult,
                op1=OP.mult,
                accum_out=kdeT[:, ci : ci + 1],
            )

        # ---- scale by e^{F}, transpose to row-major i order, DMA out ----
        ksc = const.tile([128, 32], f32)
        nc.vector.memset(ksc[:], 0.0)
        nc.vector.tensor_mul(ksc[:, 0:NCH], kdeT[:], ef[:])
        t32 = const.tile([128, 32], f32)
        nc.vector.transpose(t32[:], ksc[:])
        kout = const.tile([32, 128], f32)
        for r in range(4):
            nc.vector.tensor_copy(
                kout[0:8, r * 32 : (r + 1) * 32], t32[r * 32 : r * 32 + 8, 0:32]
            )
        nc.sync.dma_start(out=out_d[:], in_=kout[0:8, :])

    nc.compile()
    return nc


def _get_program():
    global _PROGRAM
    if _PROGRAM is None:
        _PROGRAM = _build_program()
    return _PROGRAM


def _split16(v):
    hi = v.astype(np.float16).astype(np.float64)
    return hi, v - hi


def _host_inputs(space_probs, cov_inv):
    """Per-core input maps: host-side layout + coordinate-table prep."""
    p = np.asarray(space_probs, dtype=np.float64).reshape(-1)
    A = np.asarray(cov_inv, dtype=np.float64)

    idx = np.indices(GRID, dtype=np.float64).reshape(3, N)
    cc = idx - 9.5                        # centered coords, [3, N]
    c20 = np.arange(20, dtype=np.float64) - 9.5

    G = (cc.T @ A).T                      # [3, N] g_k,i
    q = np.sum(cc * G, axis=0)            # [N]
    s = np.maximum(0.0, 9.5 * np.abs(G) - SHIFT_B)   # [3, N]
    F = -0.5 * q + s.sum(axis=0)          # [N]
    t = 0.5 * np.diag(A)[:, None] * (c20**2)[None, :]  # [3, 20]

    crossexp = -(
        A[0, 1] * cc[0] * cc[1] + A[0, 2] * cc[0] * cc[2] + A[1, 2] * cc[1] * cc[2]
    )
    wfac = np.exp(crossexp).reshape(400, 20).T
    pt = p.reshape(400, 20).T
    pw = np.zeros((20, 800), dtype=np.float32)
    pw[:, 0:400] = pt
    pw[:, 400:800] = wfac

    th = [_split16(t[k]) for k in range(3)]

    # etpat10 rows: [g1h,g1l,g2h,g2l,s1h,s1l,s2h,s2l,1(t hi),1(t lo)]
    etpat = np.zeros((10, 40), dtype=np.float16)
    etpat[0, 0:20] = c20
    etpat[1, 0:20] = c20
    etpat[4, 0:20] = -1.0
    etpat[5, 0:20] = -1.0
    etpat[8, 0:20] = -th[0][0]
    etpat[9, 0:20] = -th[0][1]
    etpat[2, 20:40] = c20
    etpat[3, 20:40] = c20
    etpat[6, 20:40] = -1.0
    etpat[7, 20:40] = -1.0
    etpat[8, 20:40] = -th[1][0]
    etpat[9, 20:40] = -th[1][1]

    # e3pat6 rows: [g3h, g3l, s3h, s3l, 1(t hi), 1(t lo)]
    e3pat = np.zeros((6, 20), dtype=np.float16)
    e3pat[0] = c20
    e3pat[1] = c20
    e3pat[2] = -1.0
    e3pat[3] = -1.0
    e3pat[4] = -th[2][0]
    e3pat[5] = -th[2][1]

    in_maps = []
    for r in range(NCORES):
        i0 = r * ROWS
        sl = slice(i0, i0 + ROWS)

        pats = np.zeros((10, _PATW), dtype=np.float16)
        hi_rows = (0, 2, 4, 6)
        lo_rows = (1, 3, 5, 7)
        for k, src in enumerate((G[0], G[1], s[0], s[1])):
            hi, lo = _split16(src[sl])
            pats[hi_rows[k], _GS0 : _GS0 + ROWS] = hi
            pats[lo_rows[k], _GS0 : _GS0 + ROWS] = lo
        pats[8, _GS0 : _GS0 + ROWS] = 1.0
        pats[9, _GS0 : _GS0 + ROWS] = 1.0

        g3h, g3l = _split16(G[2][sl])
        s3h, s3l = _split16(s[2][sl])
        pats[0, _G3 : _G3 + ROWS] = g3h
        pats[1, _G3 : _G3 + ROWS] = g3l
        pats[2, _G3 : _G3 + ROWS] = s3h
        pats[3, _G3 : _G3 + ROWS] = s3l
        pats[4, _G3 : _G3 + ROWS] = 1.0
        pats[5, _G3 : _G3 + ROWS] = 1.0

        pats[:, _ETP : _ETP + 40] = etpat
        pats[0:6, _E3P : _E3P + 20] = e3pat

        fm = np.zeros((NCH, 128), dtype=np.float32)
        fm.reshape(-1)[:ROWS] = F[sl]
        fmat = np.ascontiguousarray(fm.T)

        in_maps.append({"pw": pw, "pats": pats, "fmat": fmat})
    return in_maps


def kernel(space_probs, cov_inv):
    from concourse.bass_utils import run_bass_kernel_spmd

    nc = _get_program()
    in_maps = _host_inputs(space_probs, cov_inv)
    res = run_bass_kernel_spmd(nc, in_maps, list(range(NCORES)))
    out = np.concatenate(
        [res.results[r]["out"].reshape(-1)[:ROWS] for r in range(NCORES)]
    )
    out = out / out.sum(dtype=np.float64)
    return out.reshape(GRID).astype(np.float32)


# revision 20
# speedup vs baseline: 1.0358x; 1.0003x over previous
"""KDE on a 20^3 grid, distributed across 8 TRN2 NeuronCores.

Separable-factorization algorithm (replaces the dense 8000x8000 kernel
matrix): with A = cov_inv, q_v = v^T A v, and centered grid coords,

  kde[i] = sum_b p_b exp(-0.5(q_i + q_b - 2 GA_i . x_b))
         = e^{F_i} * sum_{x1,x2,x3} w'[x1,x2,x3] E1[x1,i] E2[x2,i] E3[x3,i]

since GA_i . x_b = sum_k g_k,i x_k,b factorizes over the tensor-product
grid.  Per-axis tables absorb t_k(x) = 0.5 A_kk x^2 (so the b-side
factor w' = p * exp(-(cross terms)) stays inside fp32 range) and per-i
shifts s_k,i = max(0, 9.5|g_k,i| - 30) (so E-table entries and partial
sums stay inside fp32 range); F_i = -0.5 q_i + sum_k s_k,i compensates.

Device pipeline per core (1000 query rows i, full b-grid, 8 i-chunks):
  - 3 packed input DMAs; exponent tiles built by tiny fp16 hi/lo
    matmuls (1-pass PE), ScalarE exponentiates to bf16/f32.
  - stage 1 is ONE bf16 matmul per chunk (k=20):
    out1[i, x12] = sum_x3 E3[x3,i] w'[x3,x12] into PSUM.
  - E12 = E1 (x) E2 prebuilt per chunk on GpSimd (SBUF-only engine).
  - main loop: PE matmul -> one DVE scalar_tensor_tensor that reads
    PSUM, multiplies by E12, and free-dim-accumulates into the kde
    column (accum_out).
  - final: multiply by e^{F}, 32x32 block-transpose, one DMA out.
Normalization (a global scalar) happens on the host after gathering.
"""

import numpy as np

GRID = (20, 20, 20)
N = 8000
NCORES = 8
ROWS = N // NCORES          # 1000 query rows per core
NCH = 8                     # i-chunks of 128 (last 24 cols are padding)
NI = NCH * 128              # 1024 padded rows per core
SHIFT_B = 30.0              # per-axis shift budget

# pats (fp16) column layout
_GS0 = 0                    # gscol10 [10, 1024]
_G3 = 1024                  # g3row6  [6, 1024]
_ETP = 2048                 # etpat10 [10, 40]
_E3P = 2088                 # e3pat6  [6, 20]
_PATW = 2108

_PROGRAM = None


def _build_program(num_devices=NCORES):
    from contextlib import ExitStack

    import concourse.bacc as bacc
    import concourse.mybir as mybir
    import concourse.tile as tile

    f32 = mybir.dt.float32
    f16 = mybir.dt.float16
    bf16 = mybir.dt.bfloat16
    OP = mybir.AluOpType
    EXP = mybir.ActivationFunctionType.Exp

    nc = bacc.Bacc(
        "TRN2",
        target_bir_lowering=False,
        debug=False,
        num_devices=num_devices,
    )

    pw_d = nc.dram_tensor("pw", [20, 800], f32, kind="ExternalInput").ap()
    pats_d = nc.dram_tensor("pats", [10, _PATW], f16, kind="ExternalInput").ap()
    fmat_d = nc.dram_tensor("fmat", [128, NCH], f32, kind="ExternalInput").ap()
    out_d = nc.dram_tensor("out", [8, 128], f32, kind="ExternalOutput").ap()

    with tile.TileContext(nc) as tc, ExitStack() as ctx:
        const = ctx.enter_context(tc.tile_pool(name="const", bufs=1))
        work = ctx.enter_context(tc.tile_pool(name="work", bufs=3))
        psum_pre = ctx.enter_context(
            tc.tile_pool(name="psum_pre", bufs=1, space="PSUM")
        )

        # ---- input loads (3 packed DMAs, dispatched from idle engines) ----
        pats_sb = const.tile([10, _PATW], f16)
        nc.gpsimd.dma_start(out=pats_sb[:], in_=pats_d[:])
        pw_sb = const.tile([20, 800], f32)
        nc.scalar.dma_start(out=pw_sb[:], in_=pw_d[:])
        fmat_sb = const.tile([128, NCH], f32)
        nc.sync.dma_start(out=fmat_sb[:], in_=fmat_d[:])

        # ---- w' = p * Wfac -> bf16 (one fused vector op) ----
        whi = const.tile([20, 400], bf16)
        nc.vector.scalar_tensor_tensor(
            whi[:], pw_sb[:, 0:400], 1.0, pw_sb[:, 400:800],
            op0=OP.mult, op1=OP.mult,
        )

        # ---- E3 [x3, i]: fp16 k=6 exponent matmuls -> Exp -> bf16 ----
        e3s = const.tile([20, NI], bf16)
        for h in range(2):
            sl = slice(h * 512, (h + 1) * 512)
            xp3 = psum_pre.tile([20, 512], f32, tag="xp3", bufs=2)
            nc.tensor.matmul(
                xp3[:],
                lhsT=pats_sb[0:6, _E3P : _E3P + 20],
                rhs=pats_sb[0:6, _G3 + h * 512 : _G3 + h * 512 + 512],
                start=True,
                stop=True,
            )
            nc.scalar.activation(e3s[:, sl], xp3[:], EXP)

        # ---- E1/E2 exponents: fp16 k=10 matmuls, packed [128, 8*40] ----
        xpe = psum_pre.tile([128, NCH * 40], f32)
        for ci in range(NCH):
            nc.tensor.matmul(
                xpe[:, ci * 40 : ci * 40 + 40],
                lhsT=pats_sb[:, ci * 128 : (ci + 1) * 128],
                rhs=pats_sb[:, _ETP : _ETP + 40],
                start=True,
                stop=True,
            )
        et = const.tile([128, NCH * 40], f32)
        nc.scalar.activation(et[:, 0 : 4 * 40], xpe[:, 0 : 4 * 40], EXP)
        nc.scalar.activation(et[:, 4 * 40 : 8 * 40], xpe[:, 4 * 40 : 8 * 40], EXP)

        # ---- e^{F} ----
        ef = const.tile([128, NCH], f32)
        nc.scalar.activation(ef[:], fmat_sb[:], EXP)

        # ---- E12 = E1 (x) E2, bf16, built on GpSimd (keeps DVE free) ----
        e12 = const.tile([128, NCH * 400], bf16)
        for ci in range(NCH):
            e1b = (
                et[:, ci * 40 : ci * 40 + 20]
                .unsqueeze(2)
                .broadcast_to((128, 20, 20))
            )
            e2b = (
                et[:, ci * 40 + 20 : ci * 40 + 40]
                .unsqueeze(1)
                .broadcast_to((128, 20, 20))
            )
            e12v = e12[:, ci * 400 : ci * 400 + 400].rearrange(
                "p (a b) -> p a b", a=20, b=20
            )
            eng = nc.vector if ci < 2 else nc.gpsimd
            eng.tensor_mul(e12v, e1b, e2b)


        # ---- main loop over 8 i-chunks ----
        kdeT = const.tile([128, NCH], f32)
        for ci in range(NCH):
            isl = slice(ci * 128, (ci + 1) * 128)
            o1p = psum_pre.tile([128, 400], f32, tag="o1p", bufs=4)
            nc.tensor.matmul(
                o1p[:], lhsT=e3s[:, isl], rhs=whi[:], start=True, stop=True
            )
            prod = work.tile([128, 400], bf16, tag="prod")
            nc.vector.scalar_tensor_tensor(
                prod[:],
                o1p[:],
                1.0,
                e12[:, ci * 400 : ci * 400 + 400],
                op0=OP.mult,
                op1=OP.mult,
                accum_out=kdeT[:, ci : ci + 1],
            )

        # ---- scale by e^{F}, transpose to row-major i order, DMA out ----
        ksc = const.tile([128, 32], f32)
        nc.vector.memset(ksc[:], 0.0)
        nc.vector.tensor_mul(ksc[:, 0:NCH], kdeT[:], ef[:])
        t32 = const.tile([128, 32], f32)
        nc.vector.transpose(t32[:], ksc[:])
        kout = const.tile([32, 128], f32)
        for r in range(4):
            nc.vector.tensor_copy(
                kout[0:8, r * 32 : (r + 1) * 32], t32[r * 32 : r * 32 + 8, 0:32]
            )
        nc.sync.dma_start(out=out_d[:], in_=kout[0:8, :])

    nc.compile()
    return nc


def _get_program():
    global _PROGRAM
    if _PROGRAM is None:
        _PROGRAM = _build_program()
    return _PROGRAM


def _split16(v):
    hi = v.astype(np.float16).astype(np.float64)
    return hi, v - hi


def _host_inputs(space_probs, cov_inv):
    """Per-core input maps: host-side layout + coordinate-table prep."""
    p = np.asarray(space_probs, dtype=np.float64).reshape(-1)
    A = np.asarray(cov_inv, dtype=np.float64)

    idx = np.indices(GRID, dtype=np.float64).reshape(3, N)
    cc = idx - 9.5                        # centered coords, [3, N]
    c20 = np.arange(20, dtype=np.float64) - 9.5

    G = (cc.T @ A).T                      # [3, N] g_k,i
    q = np.sum(cc * G, axis=0)            # [N]
    s = np.maximum(0.0, 9.5 * np.abs(G) - SHIFT_B)   # [3, N]
    F = -0.5 * q + s.sum(axis=0)          # [N]
    t = 0.5 * np.diag(A)[:, None] * (c20**2)[None, :]  # [3, 20]

    crossexp = -(
        A[0, 1] * cc[0] * cc[1] + A[0, 2] * cc[0] * cc[2] + A[1, 2] * cc[1] * cc[2]
    )
    wfac = np.exp(crossexp).reshape(400, 20).T
    pt = p.reshape(400, 20).T
    pw = np.zeros((20, 800), dtype=np.float32)
    pw[:, 0:400] = pt
    pw[:, 400:800] = wfac

    th = [_split16(t[k]) for k in range(3)]

    # etpat10 rows: [g1h,g1l,g2h,g2l,s1h,s1l,s2h,s2l,1(t hi),1(t lo)]
    etpat = np.zeros((10, 40), dtype=np.float16)
    etpat[0, 0:20] = c20
    etpat[1, 0:20] = c20
    etpat[4, 0:20] = -1.0
    etpat[5, 0:20] = -1.0
    etpat[8, 0:20] = -th[0][0]
    etpat[9, 0:20] = -th[0][1]
    etpat[2, 20:40] = c20
    etpat[3, 20:40] = c20
    etpat[6, 20:40] = -1.0
    etpat[7, 20:40] = -1.0
    etpat[8, 20:40] = -th[1][0]
    etpat[9, 20:40] = -th[1][1]

    # e3pat6 rows: [g3h, g3l, s3h, s3l, 1(t hi), 1(t lo)]
    e3pat = np.zeros((6, 20), dtype=np.float16)
    e3pat[0] = c20
    e3pat[1] = c20
    e3pat[2] = -1.0
    e3pat[3] = -1.0
    e3pat[4] = -th[2][0]
    e3pat[5] = -th[2][1]

    in_maps = []
    for r in range(NCORES):
        i0 = r * ROWS
        sl = slice(i0, i0 + ROWS)

        pats = np.zeros((10, _PATW), dtype=np.float16)
        hi_rows = (0, 2, 4, 6)
        lo_rows = (1, 3, 5, 7)
        for k, src in enumerate((G[0], G[1], s[0], s[1])):
            hi, lo = _split16(src[sl])
            pats[hi_rows[k], _GS0 : _GS0 + ROWS] = hi
            pats[lo_rows[k], _GS0 : _GS0 + ROWS] = lo
        pats[8, _GS0 : _GS0 + ROWS] = 1.0
        pats[9, _GS0 : _GS0 + ROWS] = 1.0

        g3h, g3l = _split16(G[2][sl])
        s3h, s3l = _split16(s[2][sl])
        pats[0, _G3 : _G3 + ROWS] = g3h
        pats[1, _G3 : _G3 + ROWS] = g3l
        pats[2, _G3 : _G3 + ROWS] = s3h
        pats[3, _G3 : _G3 + ROWS] = s3l
        pats[4, _G3 : _G3 + ROWS] = 1.0
        pats[5, _G3 : _G3 + ROWS] = 1.0

        pats[:, _ETP : _ETP + 40] = etpat
        pats[0:6, _E3P : _E3P + 20] = e3pat

        fm = np.zeros((NCH, 128), dtype=np.float32)
        fm.reshape(-1)[:ROWS] = F[sl]
        fmat = np.ascontiguousarray(fm.T)

        in_maps.append({"pw": pw, "pats": pats, "fmat": fmat})
    return in_maps


def kernel(space_probs, cov_inv):
    from concourse.bass_utils import run_bass_kernel_spmd

    nc = _get_program()
    in_maps = _host_inputs(space_probs, cov_inv)
    res = run_bass_kernel_spmd(nc, in_maps, list(range(NCORES)))
    out = np.concatenate(
        [res.results[r]["out"].reshape(-1)[:ROWS] for r in range(NCORES)]
    )
    out = out / out.sum(dtype=np.float64)
    return out.reshape(GRID).astype(np.float32)
